# revision 4
# baseline (speedup 1.0000x reference)
"""ContextualLoss on 8 Trainium2 NeuronCores (Bass/Tile).

Problem: nn_ContextualLoss — N=4, C=64, H=W=64, P=H*W=4096.

Math (per batch n):
  meanT    = mean of T over (N,H,W)                              [C]
  Tc/Ic    = centered features;  Tn = Tc/|Tc| per template pixel (over C)
  cos[q,p] = Ic_q . Tn_p                                         [P, P]
  mq       = max_p cos ; a2_q = 1/(1+2eps - g_q*mq), g_q = 1/|Ic_q|
  e        = exp(a2*g*cos + 1-a2) ; cs = e/sum_p e ; k_p = max_q cs
  CS_n     = mean_p k_p ;  score = mean_n(-log CS_n)

Sharding: 2 cores per batch; each core owns 2048 q rows (all 4096 p columns),
so the row min/sum are core-local. Each core outputs its partial column-max
k [128, 4096] (partition i holds max over its 16 q-blocks); host reduces.

Main loop per 128-q block, engines balanced (~6.7us each ACT/DVE):
  PE   8x matmul fp32r [128,512] into 4 PSUM tiles of [128,1024]
  ACT  drains PSUM chunks 0-2 to SBUF bf16; DVE drains chunk 3
  DVE  row max over the bf16 copy in 2 halves (2x mode), tiny scalar chain
       (dd, a2, sc, bias), then after ACT's fused exp+row-sum: rr and one
       fused scalar_tensor_tensor k = max(k, eb*rr)
Prologue: ACT tables (Square/Ln/Exp/Copy) warmed behind the 4MB t_full DMA;
meanT accum split ACT/DVE; the post-mean chain tcent->sq->h->tn runs
chunk-pipelined in 512-col slices so the first matmuls start ~3us after m.
g/h use Exp(-0.5*Ln(x)) so no Sqrt table load and no big reciprocal.
"""

import numpy as np

import concourse.bacc as bacc_mod
import concourse.mybir as mybir
import concourse.tile as tile
from concourse.bass_utils import run_bass_kernel_spmd

N, C, H, W = 4, 64, 64, 64
P = H * W                  # 4096 template pixels
QH = P // 2                # 2048 query pixels per core
NBLK = QH // 128           # 16 q-blocks per core
NCORES = 8
EPS = 1e-5
F32 = mybir.dt.float32
BF16 = mybir.dt.bfloat16
F32R = mybir.dt.float32r
AX = mybir.AxisListType
OP = mybir.AluOpType
AF = mybir.ActivationFunctionType

MM_DT = F32R       # main matmul input dtype
E_DT = BF16        # cos / ebuf / k dtype

NEG_INF = -3.0e38


def build_nc():
    nc = bacc_mod.Bacc("TRN2", target_bir_lowering=False, debug=False)

    t_full = nc.dram_tensor("t_full", [128, 2 * P], F32, kind="ExternalInput")
    t_own = nc.dram_tensor("t_own", [C, P], F32, kind="ExternalInput")
    i_own = nc.dram_tensor("i_own", [C, QH], F32, kind="ExternalInput")
    k_out = nc.dram_tensor("k_out", [128, P], E_DT, kind="ExternalOutput")

    with tile.TileContext(nc) as tc:
        with (
            tc.tile_pool(name="persist", bufs=1) as pp,
            tc.tile_pool(name="small", bufs=4) as sp,
        ):
            # ---------------- persistent tiles ----------------
            ic = pp.tile([C, QH], MM_DT)      # centered I slice (matmul lhsT)
            tn = pp.tile([C, P], MM_DT)       # normalized T (matmul rhs)
            ktile = pp.tile([128, P], E_DT)   # running column max
            g2sb = pp.tile([128, NBLK], F32)  # |Ic_q|^2 in block layout
            g = pp.tile([128, NBLK], F32)     # 1/|Ic_q|
            negg = pp.tile([128, NBLK], F32)  # -g
            onecp = pp.tile([128, 1], F32)    # 1 + 2*eps
            ones64 = pp.tile([C, 1], MM_DT)
            ones1 = pp.tile([1, C], MM_DT)

            nc.vector.memset(ktile, 0.0)
            nc.vector.memset(onecp, 1.0 + 2.0 * EPS)
            # memset can't produce fp32r; stage in f32 and copy through ACT
            ones64f = pp.tile([C, 1], F32)
            ones1f = pp.tile([1, C], F32)
            nc.vector.memset(ones64f, 1.0)
            nc.vector.memset(ones1f, 1.0)

            # ---------------- prologue ----------------
            with (
                tc.tile_pool(name="pro", bufs=1) as pro,
                tc.tile_pool(name="pps", bufs=2, space="PSUM") as pps,
            ):
                # t_full DMA first: it alone gates meanT. 4 chunks so the
                # accumulate pipelines behind the DMA.
                tfj = []
                for j in range(4):
                    t = pro.tile([128, 2048], F32, tag=f"tf{j}")
                    nc.sync.dma_start(out=t,
                                      in_=t_full[:, j * 2048:(j + 1) * 2048])
                    tfj.append(t)
                town = pro.tile([C, P], F32)
                iown = pro.tile([C, QH], F32)
                nc.sync.dma_start(out=town, in_=t_own[:, :])
                nc.sync.dma_start(out=iown, in_=i_own[:, :])

                # warm the ACT tables behind the DMA: Copy (accum below),
                # Square, Ln, Exp in first-use order.
                warm = sp.tile([1, 1], F32)
                nc.vector.memset(warm, 1.0)
                warm2 = sp.tile([1, 1], F32)
                nc.scalar.activation(out=warm2, in_=warm, func=AF.Square)
                nc.scalar.activation(out=warm2, in_=warm, func=AF.Ln)
                nc.scalar.activation(out=warm2, in_=warm, func=AF.Exp)
                nc.scalar.copy(ones64, ones64f)
                nc.scalar.copy(ones1, ones1f)

                # meanT accumulation: chunks 0,1 on ACT (copy w/ accum),
                # chunks 2,3 on DVE (reduce_sum) — both idle during the DMA.
                macc4 = sp.tile([128, 4], F32)
                for j in range(2):
                    tscj = pro.tile([128, 2048], BF16, tag="tsc")
                    nc.scalar.activation(out=tscj, in_=tfj[j], func=AF.Copy,
                                         accum_out=macc4[:, j:j + 1])
                for j in range(2, 4):
                    nc.vector.reduce_sum(out=macc4[:, j:j + 1], in_=tfj[j],
                                         axis=AX.X)
                macc = sp.tile([128, 1], F32)
                nc.vector.reduce_sum(out=macc, in_=macc4, axis=AX.X)
                # meanT[c] = (macc[c] + macc[64+c]) / 16384; bring the upper
                # half down via one small DMA, fold both subtracts into one
                # two-scalar tensor_scalar.
                ms = sp.tile([128, 1], F32)
                nc.vector.tensor_scalar_mul(ms, macc, 1.0 / (N * P))
                rot0 = sp.tile([C, 1], F32)
                nc.sync.dma_start(out=rot0, in_=ms[64:128, :])

                # g path first (block 0 needs g[:,0] before its exp):
                # ic = Ic, sqi = Ic^2, per-q sumsq via ones-matmuls,
                # g = exp(-0.5*ln(g2)) — no Sqrt table, no reciprocal.
                sqi = pro.tile([C, QH], F32)
                for c in range(2):
                    cs_ = slice(c * 1024, (c + 1) * 1024)
                    nc.vector.tensor_scalar(out=ic[:, cs_], in0=iown[:, cs_],
                                            scalar1=ms[0:C, 0:1],
                                            scalar2=rot0,
                                            op0=OP.subtract,
                                            op1=OP.subtract)
                    nc.scalar.square(sqi[:, cs_], ic[:, cs_])
                for b in range(NBLK):
                    ps = pps.tile([128, 1], F32, tag="pps")
                    nc.tensor.matmul(ps, sqi[:, b * 128:(b + 1) * 128],
                                     ones64f, start=True, stop=True)
                    nc.scalar.copy(g2sb[:, b:b + 1], ps)
                nc.scalar.activation(out=g, in_=g2sb, func=AF.Ln)
                nc.scalar.activation(out=g, in_=g, func=AF.Exp, scale=-0.5)
                nc.vector.tensor_scalar_mul(negg, g, -1.0)

                # tn chunk pipeline: tcent -> sq -> rowsum-mm -> h=e^(-ln/2)
                # -> broadcast-mm -> tn = tcent*h, in 512-col slices so the
                # main loop's first matmuls start as soon as slice 0 lands.
                tcent = pro.tile([C, P], F32)
                sqt = pro.tile([C, P], MM_DT)
                ht = pro.tile([1, P], MM_DT)
                for c in range(8):
                    cs_ = slice(c * 512, (c + 1) * 512)
                    nc.vector.tensor_scalar(out=tcent[:, cs_],
                                            in0=town[:, cs_],
                                            scalar1=ms[0:C, 0:1],
                                            scalar2=rot0,
                                            op0=OP.subtract,
                                            op1=OP.subtract)
                    nc.scalar.square(sqt[:, cs_], tcent[:, cs_])
                    ps = pps.tile([1, 512], F32, tag="pps")
                    nc.tensor.matmul(ps, ones64, sqt[:, cs_],
                                     start=True, stop=True)
                    nc.scalar.activation(out=ht[:, cs_], in_=ps, func=AF.Ln)
                    nc.scalar.activation(out=ht[:, cs_], in_=ht[:, cs_],
                                         func=AF.Exp, scale=-0.5)
                    psb = pps.tile([C, 512], F32, tag="ppsb")
                    nc.tensor.matmul(psb, ones1, ht[0:1, cs_],
                                     start=True, stop=True)
                    nc.vector.tensor_tensor(out=tn[:, cs_],
                                            in0=tcent[:, cs_], in1=psb,
                                            op=OP.mult)

            # ---------------- main loop ----------------
            NPS = 4
            PW = P // NPS
            with (
                tc.tile_pool(name="ebuf", bufs=2) as ep,
                tc.tile_pool(name="cosp", bufs=2) as cp,
                tc.tile_pool(name="mps", bufs=NPS, space="PSUM") as mps,
            ):
                for b in range(NBLK):
                    lhs = ic[:, b * 128:(b + 1) * 128]
                    eb = ep.tile([128, P], E_DT, tag="eb")
                    cos = cp.tile([128, P], E_DT, tag="cos")
                    rm2 = sp.tile([128, 2], F32, tag="rm2")
                    for h in range(NPS):
                        ps = mps.tile([128, PW], F32, tag="mps")
                        for c in range(PW // 512):
                            nc.tensor.matmul(
                                ps[:, c * 512:(c + 1) * 512], lhs,
                                tn[:, h * PW + c * 512:
                                   h * PW + (c + 1) * 512],
                                start=True, stop=True)
                        dst = cos[:, h * PW:(h + 1) * PW]
                        if h < 3:
                            nc.scalar.copy(dst, ps)
                        else:
                            nc.vector.tensor_copy(dst, ps)
                        if h == 1:
                            nc.vector.reduce_max(out=rm2[:, 0:1],
                                                 in_=cos[:, 0:2 * PW],
                                                 axis=AX.X)
                        elif h == 3:
                            nc.vector.reduce_max(out=rm2[:, 1:2],
                                                 in_=cos[:, 2 * PW:P],
                                                 axis=AX.X)
                    mq = sp.tile([128, 1], F32, tag="mq")
                    nc.vector.tensor_scalar(out=mq, in0=rm2[:, 0:1],
                                            scalar1=rm2[:, 1:2], scalar2=None,
                                            op0=OP.max)
                    dd = sp.tile([128, 1], F32, tag="dd")
                    nc.vector.scalar_tensor_tensor(
                        out=dd, in0=mq, scalar=negg[:, b:b + 1], in1=onecp,
                        op0=OP.mult, op1=OP.add)
                    a2 = sp.tile([128, 1], F32, tag="a2")
                    nc.vector.reciprocal(a2, dd)
                    sc = sp.tile([128, 1], F32, tag="sc")
                    nc.vector.tensor_tensor(out=sc, in0=a2, in1=g[:, b:b + 1],
                                            op=OP.mult)
                    bias = sp.tile([128, 1], F32, tag="bias")
                    nc.vector.tensor_scalar(out=bias, in0=a2, scalar1=-1.0,
                                            scalar2=1.0, op0=OP.mult,
                                            op1=OP.add)
                    ssum = sp.tile([128, 1], F32, tag="ssum")
                    nc.scalar.activation(out=eb, in_=cos, func=AF.Exp,
                                         bias=bias, scale=sc,
                                         accum_out=ssum)
                    rr = sp.tile([128, 1], F32, tag="rr")
                    nc.vector.reciprocal(rr, ssum)
                    # k = max(k, eb*rr): one fused stt on DVE (bf16 2x)
                    nc.vector.scalar_tensor_tensor(
                        out=ktile, in0=eb, scalar=rr, op0=OP.mult,
                        in1=ktile, op1=OP.max)

            nc.sync.dma_start(out=k_out[:, :], in_=ktile)

    nc.compile()
    return nc


_NC_CACHE = {}


def _get_nc():
    key = (str(MM_DT), str(E_DT))
    if key not in _NC_CACHE:
        _NC_CACHE[key] = build_nc()
    return _NC_CACHE[key]


def make_in_maps(I_features, T_features):
    I4 = np.ascontiguousarray(
        np.asarray(I_features, dtype=np.float32).reshape(N, C, P))
    T4 = np.ascontiguousarray(
        np.asarray(T_features, dtype=np.float32).reshape(N, C, P))
    # partition p holds flat T rows p and 128+p
    tf = np.ascontiguousarray(
        T4.reshape(2, 128, P).transpose(1, 0, 2).reshape(128, 2 * P))
    in_maps = []
    for core in range(NCORES):
        n, half = core // 2, core % 2
        in_maps.append({
            "t_full": tf,
            "t_own": np.ascontiguousarray(T4[n]),
            "i_own": np.ascontiguousarray(I4[n][:, half * QH:(half + 1) * QH]),
        })
    return in_maps


def finish_host(kparts):
    """kparts: [8, 128, P] per-core partial column maxima -> scalar score."""
    ks = np.stack([np.asarray(kp, dtype=np.float64) for kp in kparts])
    kp = ks.reshape(N, 2 * 128, P).max(axis=1)      # [N, P]
    cs = kp.mean(axis=1)                            # [N]
    return np.float32(np.mean(-np.log(cs)))


def kernel(I_features, T_features, _trace=False):
    nc = _get_nc()
    in_maps = make_in_maps(I_features, T_features)
    res = run_bass_kernel_spmd(nc, in_maps, core_ids=list(range(NCORES)),
                               trace=_trace)
    score = finish_host([r["k_out"] for r in res.results])
    if _trace:
        return np.array(score, dtype=np.float32), res
    return np.array(score, dtype=np.float32)


# revision 7
# speedup vs baseline: 1.3846x; 1.3846x over previous
"""ContextualLoss on 8 Trainium2 NeuronCores (Bass/Tile).

Problem: nn_ContextualLoss — N=4, C=64, H=W=64, P=H*W=4096.

Math (per batch n):
  meanT    = mean of T over (N,H,W)                              [C]
  Tc/Ic    = centered features;  Tn = Tc/|Tc| per template pixel (over C)
  cos[q,p] = Ic_q . Tn_p                                         [P, P]
  mq       = max_p cos ; a2_q = 1/(1+2eps - g_q*mq), g_q = 1/|Ic_q|
  e        = exp(a2*g*cos + 1-a2) ; cs = e/sum_p e ; k_p = max_q cs
  CS_n     = mean_p k_p ;  score = mean_n(-log CS_n)

Sharding: 2 cores per batch; each core owns 2048 q rows (all 4096 p columns),
so the row min/sum are core-local. Each core outputs its partial column-max
k [128, 4096] (partition i holds max over its 16 q-blocks); host reduces.

Main loop per 128-q block, engines balanced (~6.7us each ACT/DVE):
  PE   8x matmul fp32r [128,512] into 4 PSUM tiles of [128,1024]
  ACT  drains PSUM chunks 0-2 to SBUF bf16; DVE drains chunk 3
  DVE  row max over the bf16 copy in 2 halves (2x mode), tiny scalar chain
       (dd, a2, sc, bias), then after ACT's fused exp+row-sum: rr and one
       fused scalar_tensor_tensor k = max(k, eb*rr)
Prologue: ACT tables (Square/Ln/Exp/Copy) warmed behind the 4MB t_full DMA;
meanT accum split ACT/DVE; the post-mean chain tcent->sq->h->tn runs
chunk-pipelined in 512-col slices so the first matmuls start ~3us after m.
g/h use Exp(-0.5*Ln(x)) so no Sqrt table load and no big reciprocal.
"""

import numpy as np

import concourse.bacc as bacc_mod
import concourse.mybir as mybir
import concourse.tile as tile
from concourse.bass_utils import run_bass_kernel_spmd

N, C, H, W = 4, 64, 64, 64
P = H * W                  # 4096 template pixels
QH = P // 2                # 2048 query pixels per core
NBLK = QH // 128           # 16 q-blocks per core
NCORES = 8
EPS = 1e-5
F32 = mybir.dt.float32
BF16 = mybir.dt.bfloat16
F32R = mybir.dt.float32r
AX = mybir.AxisListType
OP = mybir.AluOpType
AF = mybir.ActivationFunctionType

MM_DT = F32R       # main matmul input dtype
E_DT = BF16        # cos / ebuf / k dtype

NEG_INF = -3.0e38


def build_nc():
    nc = bacc_mod.Bacc("TRN2", target_bir_lowering=False, debug=False)

    t_full = nc.dram_tensor("t_full", [128, 2 * P], F32, kind="ExternalInput")
    t_own = nc.dram_tensor("t_own", [C, P], F32, kind="ExternalInput")
    i_own = nc.dram_tensor("i_own", [C, QH], F32, kind="ExternalInput")
    k_out = nc.dram_tensor("k_out", [128, P], E_DT, kind="ExternalOutput")

    with tile.TileContext(nc) as tc:
        with (
            tc.tile_pool(name="persist", bufs=1) as pp,
            tc.tile_pool(name="small", bufs=4) as sp,
        ):
            # ---------------- persistent tiles ----------------
            ic = pp.tile([C, QH], MM_DT)      # centered I slice (matmul lhsT)
            tn = pp.tile([C, P], MM_DT)       # normalized T (matmul rhs)
            ktile = pp.tile([128, P], E_DT)   # running column max
            g2sb = pp.tile([128, NBLK], F32)  # |Ic_q|^2 in block layout
            g = pp.tile([128, NBLK], F32)     # 1/|Ic_q|
            negg = pp.tile([128, NBLK], F32)  # -g
            onecp = pp.tile([128, 1], F32)    # 1 + 2*eps
            ones64 = pp.tile([C, 1], MM_DT)
            ones1 = pp.tile([1, C], MM_DT)

            nc.vector.memset(ktile, 0.0)
            nc.vector.memset(onecp, 1.0 + 2.0 * EPS)
            # memset can't produce fp32r; stage in f32 and copy through ACT
            ones64f = pp.tile([C, 1], F32)
            ones1f = pp.tile([1, C], F32)
            nc.vector.memset(ones64f, 1.0)
            nc.vector.memset(ones1f, 1.0)

            # ---------------- prologue ----------------
            with (
                tc.tile_pool(name="pro", bufs=1) as pro,
                tc.tile_pool(name="pps", bufs=2, space="PSUM") as pps,
            ):
                # t_full DMA first: it alone gates meanT. 4 chunks so the
                # accumulate pipelines behind the DMA.
                tfj = []
                for j in range(4):
                    t = pro.tile([128, 2048], F32, tag=f"tf{j}")
                    nc.sync.dma_start(out=t,
                                      in_=t_full[:, j * 2048:(j + 1) * 2048])
                    tfj.append(t)
                town = pro.tile([C, P], F32)
                iown = pro.tile([C, QH], F32)
                nc.sync.dma_start(out=town, in_=t_own[:, :])
                nc.sync.dma_start(out=iown, in_=i_own[:, :])

                # warm the ACT tables behind the DMA: Copy (accum below),
                # Square, Ln, Exp in first-use order.
                warm = sp.tile([1, 1], F32)
                nc.vector.memset(warm, 1.0)
                warm2 = sp.tile([1, 1], F32)
                nc.scalar.activation(out=warm2, in_=warm, func=AF.Square)
                nc.scalar.activation(out=warm2, in_=warm, func=AF.Ln)
                nc.scalar.activation(out=warm2, in_=warm, func=AF.Exp)
                nc.scalar.copy(ones64, ones64f)
                nc.scalar.copy(ones1, ones1f)

                # meanT accumulation: chunks 0,1 on ACT (copy w/ accum),
                # chunks 2,3 on DVE (reduce_sum) — both idle during the DMA.
                macc4 = sp.tile([128, 4], F32)
                for j in range(2):
                    tscj = pro.tile([128, 2048], BF16, tag="tsc")
                    nc.scalar.activation(out=tscj, in_=tfj[j], func=AF.Copy,
                                         accum_out=macc4[:, j:j + 1])
                for j in range(2, 4):
                    nc.vector.reduce_sum(out=macc4[:, j:j + 1], in_=tfj[j],
                                         axis=AX.X)
                macc = sp.tile([128, 1], F32)
                nc.vector.reduce_sum(out=macc, in_=macc4, axis=AX.X)
                # meanT[c] = (macc[c] + macc[64+c]) / 16384; bring the upper
                # half down via one small DMA, fold both subtracts into one
                # two-scalar tensor_scalar.
                ms = sp.tile([128, 1], F32)
                nc.vector.tensor_scalar_mul(ms, macc, 1.0 / (N * P))
                rot0 = sp.tile([C, 1], F32)
                nc.sync.dma_start(out=rot0, in_=ms[64:128, :])

                # Post-mean chain, batched BY ACT FUNCTION so the activation
                # tables load once each (Square, Ln, Exp) instead of
                # thrashing per chunk. Within a function the work is chunked
                # so it pipelines behind the per-chunk rowsum matmuls.
                tcent = pro.tile([C, P], F32)
                sqt = pro.tile([C, P], MM_DT)
                sqi = pro.tile([C, QH], F32)
                ht = pro.tile([1, P], MM_DT)
                # centering on DVE: tn chunks first (gate the first matmuls)
                for c in range(8):
                    cs_ = slice(c * 512, (c + 1) * 512)
                    nc.vector.tensor_scalar(out=tcent[:, cs_],
                                            in0=town[:, cs_],
                                            scalar1=ms[0:C, 0:1],
                                            scalar2=rot0,
                                            op0=OP.subtract,
                                            op1=OP.subtract)
                for c in range(2):
                    cs_ = slice(c * 1024, (c + 1) * 1024)
                    nc.vector.tensor_scalar(out=ic[:, cs_], in0=iown[:, cs_],
                                            scalar1=ms[0:C, 0:1],
                                            scalar2=rot0,
                                            op0=OP.subtract,
                                            op1=OP.subtract)
                # all squares (one Square table load)
                for c in range(8):
                    cs_ = slice(c * 512, (c + 1) * 512)
                    nc.scalar.square(sqt[:, cs_], tcent[:, cs_])
                nc.scalar.square(sqi, ic)
                # |Ic_q|^2 block-layout matmuls + Copy drains (Copy still
                # warm from the meanT accum) BEFORE the Ln phase
                for b in range(NBLK):
                    ps = pps.tile([128, 1], F32, tag="gps")
                    nc.tensor.matmul(ps, sqi[:, b * 128:(b + 1) * 128],
                                     ones64f, start=True, stop=True)
                    nc.scalar.copy(g2sb[:, b:b + 1], ps)
                # |Tc_p|^2 rowsum matmul + Ln per chunk (one Ln table load),
                # then all Exp(-x/2): rsqrt with no Sqrt table
                for c in range(8):
                    cs_ = slice(c * 512, (c + 1) * 512)
                    ps = pps.tile([1, 512], F32, tag="pps")
                    nc.tensor.matmul(ps, ones64, sqt[:, cs_],
                                     start=True, stop=True)
                    nc.scalar.activation(out=ht[:, cs_], in_=ps, func=AF.Ln)
                nc.scalar.activation(out=g2sb, in_=g2sb, func=AF.Ln)
                for c in range(8):
                    nc.scalar.activation(out=ht[:, c * 512:(c + 1) * 512],
                                         in_=ht[:, c * 512:(c + 1) * 512],
                                         func=AF.Exp, scale=-0.5)
                nc.scalar.activation(out=g, in_=g2sb, func=AF.Exp,
                                     scale=-0.5)
                nc.vector.tensor_scalar_mul(negg, g, -1.0)
                # broadcast h to 64 partitions and scale tn chunks
                for c in range(8):
                    cs_ = slice(c * 512, (c + 1) * 512)
                    psb = pps.tile([C, 512], F32, tag=f"ppsb{c % 2}")
                    nc.tensor.matmul(psb, ones1, ht[0:1, cs_],
                                     start=True, stop=True)
                    nc.vector.tensor_tensor(out=tn[:, cs_],
                                            in0=tcent[:, cs_], in1=psb,
                                            op=OP.mult)

            # ---------------- main loop ----------------
            # PSUM as 2x[128,2048] tiles (4 banks each) drained by ACT
            # copies; DVE finds the row max with a bf16 tensor_tensor max
            # tree (tt runs 2x on bf16, reduce only 1x) and accumulates
            # k with tensor_scalar (4x) + tensor_tensor (2x).
            NPS = 2
            PW = P // NPS
            with (
                tc.tile_pool(name="ebuf", bufs=2) as ep,
                tc.tile_pool(name="cosp", bufs=2) as cp,
                tc.tile_pool(name="tree", bufs=2) as tp,
                tc.tile_pool(name="mps", bufs=NPS, space="PSUM") as mps,
            ):
                for b in range(NBLK):
                    lhs = ic[:, b * 128:(b + 1) * 128]
                    eb = ep.tile([128, P], E_DT, tag="eb")
                    cos = cp.tile([128, P], E_DT, tag="cos")
                    for h in range(NPS):
                        ps = mps.tile([128, PW], F32, tag="mps")
                        for c in range(PW // 512):
                            nc.tensor.matmul(
                                ps[:, c * 512:(c + 1) * 512], lhs,
                                tn[:, h * PW + c * 512:
                                   h * PW + (c + 1) * 512],
                                start=True, stop=True)
                        nc.scalar.copy(cos[:, h * PW:(h + 1) * PW], ps)
                    # bf16 max tree: 4096 -> 2048 -> 1024 -> 512 -> mq
                    s1 = tp.tile([128, 2048], E_DT, tag="s1")
                    nc.vector.tensor_tensor(out=s1, in0=cos[:, 0:2048],
                                            in1=cos[:, 2048:4096],
                                            op=OP.max)
                    s2 = tp.tile([128, 1024], E_DT, tag="s2")
                    nc.vector.tensor_tensor(out=s2, in0=s1[:, 0:1024],
                                            in1=s1[:, 1024:2048],
                                            op=OP.max)
                    s3 = tp.tile([128, 512], E_DT, tag="s3")
                    nc.vector.tensor_tensor(out=s3, in0=s2[:, 0:512],
                                            in1=s2[:, 512:1024],
                                            op=OP.max)
                    mq = sp.tile([128, 1], F32, tag="mq")
                    nc.vector.reduce_max(out=mq, in_=s3, axis=AX.X)
                    dd = sp.tile([128, 1], F32, tag="dd")
                    nc.vector.scalar_tensor_tensor(
                        out=dd, in0=mq, scalar=negg[:, b:b + 1], in1=onecp,
                        op0=OP.mult, op1=OP.add)
                    a2 = sp.tile([128, 1], F32, tag="a2")
                    nc.vector.reciprocal(a2, dd)
                    sc = sp.tile([128, 1], F32, tag="sc")
                    nc.vector.tensor_tensor(out=sc, in0=a2, in1=g[:, b:b + 1],
                                            op=OP.mult)
                    bias = sp.tile([128, 1], F32, tag="bias")
                    nc.vector.tensor_scalar(out=bias, in0=a2, scalar1=-1.0,
                                            scalar2=1.0, op0=OP.mult,
                                            op1=OP.add)
                    ssum = sp.tile([128, 1], F32, tag="ssum")
                    nc.scalar.activation(out=eb, in_=cos, func=AF.Exp,
                                         bias=bias, scale=sc,
                                         accum_out=ssum)
                    rr = sp.tile([128, 1], F32, tag="rr")
                    nc.vector.reciprocal(rr, ssum)
                    # k = max(k, eb*rr): ts (4x) + tt (2x)
                    csb = ep.tile([128, P], E_DT, tag="csb")
                    nc.vector.tensor_scalar(out=csb, in0=eb, scalar1=rr,
                                            scalar2=None, op0=OP.mult)
                    nc.vector.tensor_tensor(out=ktile, in0=ktile, in1=csb,
                                            op=OP.max)

            nc.sync.dma_start(out=k_out[:, :], in_=ktile)

    nc.compile()
    return nc


_NC_CACHE = {}


def _get_nc():
    key = (str(MM_DT), str(E_DT))
    if key not in _NC_CACHE:
        _NC_CACHE[key] = build_nc()
    return _NC_CACHE[key]


def make_in_maps(I_features, T_features):
    I4 = np.ascontiguousarray(
        np.asarray(I_features, dtype=np.float32).reshape(N, C, P))
    T4 = np.ascontiguousarray(
        np.asarray(T_features, dtype=np.float32).reshape(N, C, P))
    # partition p holds flat T rows p and 128+p
    tf = np.ascontiguousarray(
        T4.reshape(2, 128, P).transpose(1, 0, 2).reshape(128, 2 * P))
    in_maps = []
    for core in range(NCORES):
        n, half = core // 2, core % 2
        in_maps.append({
            "t_full": tf,
            "t_own": np.ascontiguousarray(T4[n]),
            "i_own": np.ascontiguousarray(I4[n][:, half * QH:(half + 1) * QH]),
        })
    return in_maps


def finish_host(kparts):
    """kparts: [8, 128, P] per-core partial column maxima -> scalar score."""
    ks = np.stack([np.asarray(kp, dtype=np.float64) for kp in kparts])
    kp = ks.reshape(N, 2 * 128, P).max(axis=1)      # [N, P]
    cs = kp.mean(axis=1)                            # [N]
    return np.float32(np.mean(-np.log(cs)))


def kernel(I_features, T_features, _trace=False):
    nc = _get_nc()
    in_maps = make_in_maps(I_features, T_features)
    res = run_bass_kernel_spmd(nc, in_maps, core_ids=list(range(NCORES)),
                               trace=_trace)
    score = finish_host([r["k_out"] for r in res.results])
    if _trace:
        return np.array(score, dtype=np.float32), res
    return np.array(score, dtype=np.float32)
